# revision 43
# baseline (speedup 1.0000x reference)
"""CCAMDec cross-channel attention kernel for Trainium2 (Bass/Tile).

Per batch b (8 batches, one per NeuronCore, data-parallel):
    energy = X @ Y^T            [C=512, K=512], contract N=4096
    attn   = softmax(max(energy) - energy)  == softmax(-energy)   (rows)
    out    = x + scale * (attn @ Y)         [C, N]

Layout strategy per core:
  - x, y loaded resident in SBUF as 4 chunks [128, 4096] each.
  - Phase 1: for each n-chunk t (32 x 128): PE-transpose x/y column slices
    into xT_t/yT_t [128n, 512], then 4 accumulating matmuls (fp32r,
    moving free dim 512) build energy in 4 PSUM banks.
  - Softmax over free dim K: min-reduce (softmax(-E) stabilized with
    min(E)), exp via ACT with fused row-sum accum, reciprocal, and the
    runtime `scale` folded into the normalization.
  - attn transposed (16 PE transposes) to attT [K, C] = stationary for
    phase 2; matmul 2 uses natural-layout y as the moving operand.
  - Phase 2: out[cb, ns] = x + psum(attT.T @ y), DVE add, DMA out.
"""

import numpy as np

import concourse.bass as bass
import concourse.bass_utils as _bu
import concourse.mybir as mybir
import concourse.tile as tile
from concourse.bass_utils import run_bass_kernel_spmd
from concourse.masks import make_identity

# Enable walrus LDWEIGHTS dedup (measured ~2us win, output identical).
if not getattr(_bu.run_command, "_ldwopt_patched", False):
    _orig_run_command = _bu.run_command

    def _run_command_ldwopt(argv, **kwargs):
        argv = [
            a.replace("--enable-ldw-opt=false", "--enable-ldw-opt=true")
            if isinstance(a, str)
            else a
            for a in argv
        ]
        return _orig_run_command(argv, **kwargs)

    _run_command_ldwopt._ldwopt_patched = True
    _bu.run_command = _run_command_ldwopt

B, C, K, W, H = 8, 512, 512, 64, 64
N = W * H  # 4096
P = 128
CB = C // P  # 4 chunks of channels
KB = K // P  # 4 chunks of keys
NT = N // P  # 32 n-chunks (transpose granularity)
NS = N // 512  # 8 output column tiles

FP32 = mybir.dt.float32
F32R = mybir.dt.float32r

# Big-matmul operand dtype: float32r streams at full PE rate (1 cyc/row at
# free dim >= 256) vs float32's 4 cyc/row. Bitcast only; bits are fp32.
MM_DT = F32R


def _split_ctrl_waits(m, maxw=1):
    """This walrus build accepts only one sync wait per instruction encoding.
    Move excess waits onto injected NoOps just before the instruction (same
    engine queue, so ordering semantics are preserved)."""
    n = 0
    for fn in m.functions:
        for bb in fn.blocks:
            new = []
            for inst in bb.instructions:
                si = inst.sync_info
                if si is not None and si.on_wait and len(si.on_wait) > maxw:
                    waits = list(si.on_wait)
                    extra, keep = waits[:-maxw], waits[-maxw:]
                    for i in range(0, len(extra), maxw):
                        new.append(
                            mybir.InstNoOp(
                                name=f"{inst.name}-ws{i}",
                                engine=inst.engine,
                                ins=[],
                                outs=[],
                                sync_info=mybir.SyncInfo(
                                    on_wait=extra[i : i + maxw], on_update=[]
                                ),
                            )
                        )
                        n += 1
                    si.on_wait = keep
                new.append(inst)
            bb.instructions = new
    return n


def build_nc(split_ctrl_waits=True):
    nc = bass.Bass()
    x_in = nc.dram_tensor("x", [C, N], FP32, kind="ExternalInput")
    y_in = nc.dram_tensor("y", [K, N], FP32, kind="ExternalInput")
    s_in = nc.dram_tensor("scale", [1, 1], FP32, kind="ExternalInput")
    out = nc.dram_tensor("out", [C, N], FP32, kind="ExternalOutput")

    with tile.TileContext(nc) as tc:
        with (
            tc.tile_pool(name="const", bufs=1) as const,
            tc.tile_pool(name="resident", bufs=1) as res,
            tc.tile_pool(name="work", bufs=4) as work,
            tc.tile_pool(name="psum_e", bufs=1, space="PSUM") as psum_e,
            tc.tile_pool(name="psum_w", bufs=4, space="PSUM") as psum_w,
        ):
            ident = const.tile([P, P], FP32)
            make_identity(nc, ident)

            # PE prewarm: ~4us of junk transposes while the first DMA slices
            # land. HAM needs ~3.4us of sustained PE activity to unthrottle
            # (1.2 -> 2.4 GHz); without this the first ~15us of real matmuls
            # run at half clock.
            warm_ps = psum_w.tile([P, 512], FP32, tag="work", name="warm_ps")
            for w in range(28):
                nc.tensor.transpose(
                    warm_ps[:, (w % 4) * P : (w % 4 + 1) * P], ident, ident
                )

            ones = const.tile([1, P], FP32)
            nc.vector.memset(ones, 1.0)
            scale_sb = const.tile([1, 1], FP32)
            nc.sync.dma_start(scale_sb, s_in[:])
            # broadcast scale across partitions: [128,1] = ones.T @ scale
            scale_ps = psum_w.tile([P, 512], FP32, tag="work")
            nc.tensor.matmul(
                scale_ps[:, :1], lhsT=ones, rhs=scale_sb, start=True, stop=True
            )
            scale_bc = const.tile([P, 1], FP32)
            nc.vector.tensor_copy(scale_bc, scale_ps[:, :1])

            x_sb = [res.tile([P, N], FP32, name=f"x{cb}") for cb in range(CB)]
            # y doubles as the phase-2 moving operand, so it lives as f32r;
            # the DMA moves raw fp32 bits (PE truncates mantissa on read).
            y_sb = [res.tile([P, N], F32R, name=f"y{kb}") for kb in range(KB)]
            # interleave loads n-slice-major so phase 1 can start early;
            # the first two slices are small so the first transposes start asap
            # x loads dispatch from the SP HWDGE queue, y loads from the ACT
            # HWDGE queue — parallel dispatch halves time-to-first-transpose.
            bounds = [0, 128, 512, 1024, 2048, 3072, 4096]
            for s in range(len(bounds) - 1):
                ssl = slice(bounds[s], bounds[s + 1])
                for cb in range(CB):
                    nc.sync.dma_start(
                        x_sb[cb][:, ssl], x_in[cb * P : (cb + 1) * P, ssl]
                    )
                for kb in range(KB):
                    # first y slices dispatch from the (otherwise idle) SWDGE
                    # queue so the 8 t=0 prerequisites issue in parallel
                    eng = nc.gpsimd if s == 0 else nc.sync
                    eng.dma_start(
                        y_sb[kb][:, ssl],
                        y_in[kb * P : (kb + 1) * P, ssl].bitcast(F32R),
                    )

            # ---- phase 1: energy = X @ Y^T, accumulated over 32 n-chunks
            energy_ps = [
                psum_e.tile([P, 512], FP32, name=f"energy{cb}") for cb in range(CB)
            ]
            for t in range(NT):
                tsl = slice(t * P, (t + 1) * P)
                xT_ps = psum_w.tile([P, 512], FP32, tag="work")
                for cb in range(CB):
                    nc.tensor.transpose(
                        xT_ps[:, cb * P : (cb + 1) * P], x_sb[cb][:, tsl], ident
                    )
                xT_sb = work.tile([P, 512], MM_DT, tag="xT")
                nc.vector.tensor_copy(xT_sb, xT_ps)

                yT_ps = psum_w.tile([P, 512], FP32, tag="work")
                for kb in range(KB):
                    nc.tensor.transpose(
                        yT_ps[:, kb * P : (kb + 1) * P],
                        y_sb[kb][:, tsl].bitcast(FP32),
                        ident,
                    )
                yT_sb = work.tile([P, 512], MM_DT, tag="yT")
                nc.vector.tensor_copy(yT_sb, yT_ps)

                for cb in range(CB):
                    nc.tensor.matmul(
                        energy_ps[cb],
                        lhsT=xT_sb[:, cb * P : (cb + 1) * P],
                        rhs=yT_sb,
                        start=(t == 0),
                        stop=(t == NT - 1),
                        skip_group_check=True,
                    )

            # ---- softmax over K (free dim). softmax(max-E) == softmax(-E);
            # stabilized: exp(min(E) - E) / sum. Runtime scale folded in.
            # attn chunks transpose into per-kb PSUM right after each row
            # softmax so the phase-2 stationary tiles land early.
            att_sb = [res.tile([P, 512], FP32, name=f"att{cb}") for cb in range(CB)]
            attT_ps = [
                psum_w.tile([P, 512], FP32, tag="work", name=f"attTps{kb}")
                for kb in range(KB)
            ]
            # normalization (1/rowsum * scale) is deferred to phase 2, where
            # it rides on the output rows (same partition layout); this keeps
            # the softmax -> transpose chain short so PE stays warm.
            rs_sb = [res.tile([P, 1], FP32, name=f"rs{cb}") for cb in range(CB)]
            for cb in range(CB):
                mn = work.tile([P, 1], FP32, tag="mn")
                nc.vector.tensor_reduce(
                    mn,
                    energy_ps[cb],
                    axis=mybir.AxisListType.X,
                    op=mybir.AluOpType.min,
                )
                ssum = work.tile([P, 1], FP32, tag="ssum")
                nc.scalar.activation(
                    att_sb[cb],
                    energy_ps[cb],
                    mybir.ActivationFunctionType.Exp,
                    bias=mn,
                    scale=-1.0,
                    accum_out=ssum,
                )
                for kb in range(KB):
                    nc.tensor.transpose(
                        attT_ps[kb][:, cb * P : (cb + 1) * P],
                        att_sb[cb][:, kb * P : (kb + 1) * P],
                        ident,
                    )
                nc.vector.reciprocal(rs_sb[cb], ssum)
                nc.vector.tensor_tensor(
                    rs_sb[cb], rs_sb[cb], scale_bc, mybir.AluOpType.mult
                )
            attT_sb = [res.tile([P, 512], MM_DT, name=f"attT{kb}") for kb in range(KB)]
            for kb in range(KB):
                nc.vector.tensor_copy(attT_sb[kb], attT_ps[kb])

            # ---- phase 2: out = x + (scaled attn) @ Y
            # k-outer per cb with 8 open PSUM banks: each attT stationary is
            # reused across 8 consecutive matmuls (weight reload amortized).
            for cb in range(CB):
                ps2 = []
                for ns in range(NS):
                    if ns < 4:
                        ps2.append(psum_e.tile([P, 512], FP32, name=f"energy{ns}"))
                    else:
                        ps2.append(
                            psum_w.tile(
                                [P, 512], FP32, tag="work", name=f"o{cb}_{ns}"
                            )
                        )
                # per-tile kb-inner so bank drains spread across the block;
                # adjacent ns pairs share one [128,1024] store to halve the
                # store-dispatch count on the sync queue
                o_sb = None
                for ns in range(NS):
                    for kb in range(KB):
                        nc.tensor.matmul(
                            ps2[ns],
                            lhsT=attT_sb[kb][:, cb * P : (cb + 1) * P],
                            rhs=y_sb[kb][:, ns * 512 : (ns + 1) * 512],
                            start=(kb == 0),
                            stop=(kb == KB - 1),
                            skip_group_check=True,
                        )
                    # drain this bank: normalize on ACT (1/rowsum * scale),
                    # residual on DVE, store pairs
                    nsl = slice(ns * 512, (ns + 1) * 512)
                    t_sb = work.tile([P, 512], FP32, tag="tsb")
                    nc.scalar.activation(
                        t_sb,
                        ps2[ns],
                        mybir.ActivationFunctionType.Copy,
                        scale=rs_sb[cb],
                    )
                    if ns % 2 == 0:
                        o_sb = work.tile([P, 1024], FP32, tag="osb", name="o_sb")
                    half = slice((ns % 2) * 512, (ns % 2) * 512 + 512)
                    nc.vector.tensor_tensor(
                        o_sb[:, half], x_sb[cb][:, nsl], t_sb, mybir.AluOpType.add
                    )
                    if ns % 2 == 1:
                        osl = slice((ns - 1) * 512, (ns + 1) * 512)
                        nc.sync.dma_start(out[cb * P : (cb + 1) * P, osl], o_sb)

    if split_ctrl_waits:
        _split_ctrl_waits(nc.m)
    return nc


_NC_CACHE = []


def kernel(x, y, scale):
    if not _NC_CACHE:
        _NC_CACHE.append(build_nc())
    nc = _NC_CACHE[0]
    x = np.ascontiguousarray(x, dtype=np.float32).reshape(B, C, N)
    y = np.ascontiguousarray(y, dtype=np.float32).reshape(B, K, N)
    s = np.ascontiguousarray(scale, dtype=np.float32).reshape(1, 1)
    in_maps = [{"x": x[b], "y": y[b], "scale": s} for b in range(B)]
    res = run_bass_kernel_spmd(nc, in_maps, list(range(B)))
    outs = np.stack([res.results[b]["out"] for b in range(B)])
    return outs.reshape(B, C, W, H).astype(np.float32)


# revision 46
# speedup vs baseline: 1.0092x; 1.0092x over previous
"""CCAMDec cross-channel attention kernel for Trainium2 (Bass/Tile).

Per batch b (8 batches, one per NeuronCore, data-parallel):
    energy = X @ Y^T            [C=512, K=512], contract N=4096
    attn   = softmax(max(energy) - energy)  == softmax(-energy)   (rows)
    out    = x + scale * (attn @ Y)         [C, N]

Layout strategy per core:
  - x, y loaded resident in SBUF as 4 chunks [128, 4096] each.
  - Phase 1: for each n-chunk t (32 x 128): PE-transpose x/y column slices
    into xT_t/yT_t [128n, 512], then 4 accumulating matmuls (fp32r,
    moving free dim 512) build energy in 4 PSUM banks.
  - Softmax over free dim K: min-reduce (softmax(-E) stabilized with
    min(E)), exp via ACT with fused row-sum accum, reciprocal, and the
    runtime `scale` folded into the normalization.
  - attn transposed (16 PE transposes) to attT [K, C] = stationary for
    phase 2; matmul 2 uses natural-layout y as the moving operand.
  - Phase 2: out[cb, ns] = x + psum(attT.T @ y), DVE add, DMA out.
"""

import numpy as np

import concourse.bass as bass
import concourse.bass_utils as _bu
import concourse.mybir as mybir
import concourse.tile as tile
from concourse.bass_utils import run_bass_kernel_spmd
from concourse.masks import make_identity

# Enable walrus LDWEIGHTS dedup (measured ~2us win, output identical).
if not getattr(_bu.run_command, "_ldwopt_patched", False):
    _orig_run_command = _bu.run_command

    def _run_command_ldwopt(argv, **kwargs):
        argv = [
            a.replace("--enable-ldw-opt=false", "--enable-ldw-opt=true")
            if isinstance(a, str)
            else a
            for a in argv
        ]
        return _orig_run_command(argv, **kwargs)

    _run_command_ldwopt._ldwopt_patched = True
    _bu.run_command = _run_command_ldwopt

B, C, K, W, H = 8, 512, 512, 64, 64
N = W * H  # 4096
P = 128
CB = C // P  # 4 chunks of channels
KB = K // P  # 4 chunks of keys
NT = N // P  # 32 n-chunks (transpose granularity)
NS = N // 512  # 8 output column tiles

FP32 = mybir.dt.float32
F32R = mybir.dt.float32r

# Big-matmul operand dtype: float32r streams at full PE rate (1 cyc/row at
# free dim >= 256) vs float32's 4 cyc/row. Bitcast only; bits are fp32.
MM_DT = F32R


def _split_ctrl_waits(m, maxw=1):
    """This walrus build accepts only one sync wait per instruction encoding.
    Move excess waits onto injected NoOps just before the instruction (same
    engine queue, so ordering semantics are preserved)."""
    n = 0
    for fn in m.functions:
        for bb in fn.blocks:
            new = []
            for inst in bb.instructions:
                si = inst.sync_info
                if si is not None and si.on_wait and len(si.on_wait) > maxw:
                    waits = list(si.on_wait)
                    extra, keep = waits[:-maxw], waits[-maxw:]
                    for i in range(0, len(extra), maxw):
                        new.append(
                            mybir.InstNoOp(
                                name=f"{inst.name}-ws{i}",
                                engine=inst.engine,
                                ins=[],
                                outs=[],
                                sync_info=mybir.SyncInfo(
                                    on_wait=extra[i : i + maxw], on_update=[]
                                ),
                            )
                        )
                        n += 1
                    si.on_wait = keep
                new.append(inst)
            bb.instructions = new
    return n


def build_nc(split_ctrl_waits=True):
    nc = bass.Bass()
    x_in = nc.dram_tensor("x", [C, N], FP32, kind="ExternalInput")
    y_in = nc.dram_tensor("y", [K, N], FP32, kind="ExternalInput")
    s_in = nc.dram_tensor("scale", [1, 1], FP32, kind="ExternalInput")
    ident_in = nc.dram_tensor("ident", [P, P], FP32, kind="ExternalInput")
    out = nc.dram_tensor("out", [C, N], FP32, kind="ExternalOutput")

    with tile.TileContext(nc) as tc:
        with (
            tc.tile_pool(name="const", bufs=1) as const,
            tc.tile_pool(name="resident", bufs=1) as res,
            tc.tile_pool(name="work", bufs=4) as work,
            tc.tile_pool(name="psum_e", bufs=1, space="PSUM") as psum_e,
            tc.tile_pool(name="psum_w", bufs=4, space="PSUM") as psum_w,
        ):
            # identity comes from DRAM (first dispatch on the sync queue) so
            # neither gpsimd iota latency nor load dispatches gate the prewarm
            ident = const.tile([P, P], FP32)
            nc.sync.dma_start(ident, ident_in[:])

            # PE prewarm: ~4us of junk transposes while the first DMA slices
            # land. HAM needs ~3.4us of sustained PE activity to unthrottle
            # (1.2 -> 2.4 GHz); without this the first ~15us of real matmuls
            # run at half clock.
            warm_ps = psum_w.tile([P, 512], FP32, tag="work", name="warm_ps")
            for w in range(28):
                nc.tensor.transpose(
                    warm_ps[:, (w % 4) * P : (w % 4 + 1) * P], ident, ident
                )

            ones = const.tile([1, P], FP32)
            nc.vector.memset(ones, 1.0)
            scale_sb = const.tile([1, 1], FP32)
            nc.sync.dma_start(scale_sb, s_in[:])
            # broadcast scale across partitions: [128,1] = ones.T @ scale
            scale_ps = psum_w.tile([P, 512], FP32, tag="work")
            nc.tensor.matmul(
                scale_ps[:, :1], lhsT=ones, rhs=scale_sb, start=True, stop=True
            )
            scale_bc = const.tile([P, 1], FP32)
            nc.vector.tensor_copy(scale_bc, scale_ps[:, :1])

            x_sb = [res.tile([P, N], FP32, name=f"x{cb}") for cb in range(CB)]
            # y doubles as the phase-2 moving operand, so it lives as f32r;
            # the DMA moves raw fp32 bits (PE truncates mantissa on read).
            y_sb = [res.tile([P, N], F32R, name=f"y{kb}") for kb in range(KB)]
            # interleave loads n-slice-major so phase 1 can start early;
            # the first two slices are small so the first transposes start asap
            # x loads dispatch from the SP HWDGE queue, y loads from the ACT
            # HWDGE queue — parallel dispatch halves time-to-first-transpose.
            bounds = [0, 128, 512, 1024, 2048, 3072, 4096]
            for s in range(len(bounds) - 1):
                ssl = slice(bounds[s], bounds[s + 1])
                for cb in range(CB):
                    nc.sync.dma_start(
                        x_sb[cb][:, ssl], x_in[cb * P : (cb + 1) * P, ssl]
                    )
                for kb in range(KB):
                    # first y slices dispatch from the (otherwise idle) SWDGE
                    # queue so the 8 t=0 prerequisites issue in parallel
                    eng = nc.gpsimd if s == 0 else nc.sync
                    eng.dma_start(
                        y_sb[kb][:, ssl],
                        y_in[kb * P : (kb + 1) * P, ssl].bitcast(F32R),
                    )

            # ---- phase 1: energy = X @ Y^T, accumulated over 32 n-chunks
            energy_ps = [
                psum_e.tile([P, 512], FP32, name=f"energy{cb}") for cb in range(CB)
            ]
            for t in range(NT):
                tsl = slice(t * P, (t + 1) * P)
                xT_ps = psum_w.tile([P, 512], FP32, tag="work")
                for cb in range(CB):
                    nc.tensor.transpose(
                        xT_ps[:, cb * P : (cb + 1) * P], x_sb[cb][:, tsl], ident
                    )
                xT_sb = work.tile([P, 512], MM_DT, tag="xT")
                nc.vector.tensor_copy(xT_sb, xT_ps)

                yT_ps = psum_w.tile([P, 512], FP32, tag="work")
                for kb in range(KB):
                    nc.tensor.transpose(
                        yT_ps[:, kb * P : (kb + 1) * P],
                        y_sb[kb][:, tsl].bitcast(FP32),
                        ident,
                    )
                yT_sb = work.tile([P, 512], MM_DT, tag="yT")
                nc.vector.tensor_copy(yT_sb, yT_ps)

                for cb in range(CB):
                    nc.tensor.matmul(
                        energy_ps[cb],
                        lhsT=xT_sb[:, cb * P : (cb + 1) * P],
                        rhs=yT_sb,
                        start=(t == 0),
                        stop=(t == NT - 1),
                        skip_group_check=True,
                    )

            # ---- softmax over K (free dim). softmax(max-E) == softmax(-E);
            # stabilized: exp(min(E) - E) / sum. Runtime scale folded in.
            # attn chunks transpose into per-kb PSUM right after each row
            # softmax so the phase-2 stationary tiles land early.
            att_sb = [res.tile([P, 512], FP32, name=f"att{cb}") for cb in range(CB)]
            attT_ps = [
                psum_w.tile([P, 512], FP32, tag="work", name=f"attTps{kb}")
                for kb in range(KB)
            ]
            # normalization (1/rowsum * scale) is deferred to phase 2, where
            # it rides on the output rows (same partition layout); this keeps
            # the softmax -> transpose chain short so PE stays warm.
            rs_sb = [res.tile([P, 1], FP32, name=f"rs{cb}") for cb in range(CB)]
            for cb in range(CB):
                mn = work.tile([P, 1], FP32, tag="mn")
                nc.vector.tensor_reduce(
                    mn,
                    energy_ps[cb],
                    axis=mybir.AxisListType.X,
                    op=mybir.AluOpType.min,
                )
                ssum = work.tile([P, 1], FP32, tag="ssum")
                nc.scalar.activation(
                    att_sb[cb],
                    energy_ps[cb],
                    mybir.ActivationFunctionType.Exp,
                    bias=mn,
                    scale=-1.0,
                    accum_out=ssum,
                )
                for kb in range(KB):
                    nc.tensor.transpose(
                        attT_ps[kb][:, cb * P : (cb + 1) * P],
                        att_sb[cb][:, kb * P : (kb + 1) * P],
                        ident,
                    )
                nc.vector.reciprocal(rs_sb[cb], ssum)
                nc.vector.tensor_tensor(
                    rs_sb[cb], rs_sb[cb], scale_bc, mybir.AluOpType.mult
                )
            attT_sb = [res.tile([P, 512], MM_DT, name=f"attT{kb}") for kb in range(KB)]
            for kb in range(KB):
                nc.vector.tensor_copy(attT_sb[kb], attT_ps[kb])

            # ---- phase 2: out = x + (scaled attn) @ Y
            # k-outer per cb with 8 open PSUM banks: each attT stationary is
            # reused across 8 consecutive matmuls (weight reload amortized).
            for cb in range(CB):
                ps2 = []
                for ns in range(NS):
                    if ns < 4:
                        ps2.append(psum_e.tile([P, 512], FP32, name=f"energy{ns}"))
                    else:
                        ps2.append(
                            psum_w.tile(
                                [P, 512], FP32, tag="work", name=f"o{cb}_{ns}"
                            )
                        )
                # per-tile kb-inner so bank drains spread across the block;
                # adjacent ns pairs share one [128,1024] store to halve the
                # store-dispatch count on the sync queue
                o_sb = None
                for ns in range(NS):
                    for kb in range(KB):
                        nc.tensor.matmul(
                            ps2[ns],
                            lhsT=attT_sb[kb][:, cb * P : (cb + 1) * P],
                            rhs=y_sb[kb][:, ns * 512 : (ns + 1) * 512],
                            start=(kb == 0),
                            stop=(kb == KB - 1),
                            skip_group_check=True,
                        )
                    # drain this bank: normalize on ACT (1/rowsum * scale),
                    # residual on DVE, store pairs
                    nsl = slice(ns * 512, (ns + 1) * 512)
                    t_sb = work.tile([P, 512], FP32, tag="tsb")
                    nc.scalar.activation(
                        t_sb,
                        ps2[ns],
                        mybir.ActivationFunctionType.Copy,
                        scale=rs_sb[cb],
                    )
                    if ns % 2 == 0:
                        o_sb = work.tile([P, 1024], FP32, tag="osb", name="o_sb")
                    half = slice((ns % 2) * 512, (ns % 2) * 512 + 512)
                    nc.vector.tensor_tensor(
                        o_sb[:, half], x_sb[cb][:, nsl], t_sb, mybir.AluOpType.add
                    )
                    if ns % 2 == 1:
                        osl = slice((ns - 1) * 512, (ns + 1) * 512)
                        nc.sync.dma_start(out[cb * P : (cb + 1) * P, osl], o_sb)

    if split_ctrl_waits:
        _split_ctrl_waits(nc.m)
    return nc


_NC_CACHE = []


def kernel(x, y, scale):
    if not _NC_CACHE:
        _NC_CACHE.append(build_nc())
    nc = _NC_CACHE[0]
    x = np.ascontiguousarray(x, dtype=np.float32).reshape(B, C, N)
    y = np.ascontiguousarray(y, dtype=np.float32).reshape(B, K, N)
    s = np.ascontiguousarray(scale, dtype=np.float32).reshape(1, 1)
    ident = np.eye(P, dtype=np.float32)
    in_maps = [
        {"x": x[b], "y": y[b], "scale": s, "ident": ident} for b in range(B)
    ]
    res = run_bass_kernel_spmd(nc, in_maps, list(range(B)))
    outs = np.stack([res.results[b]["out"] for b in range(B)])
    return outs.reshape(B, C, W, H).astype(np.float32)
